# revision 1
# baseline (speedup 1.0000x reference)
"""Trainium2 Bass kernel for nn_MultiHeadSelfAttention_55654186222044.

Reference math (per batch b, per "slice" h of the reshaped activations):
    xs  = x[b,:,h*64:(h+1)*64]                  (T=1024, D=64)
    q_i = xs @ Wq[i].T + bq[i]   (per param set i=0..15), same k_i, v_i
    scores_i = q_i.T @ k_i / 8   (64x64, contraction over T!)
    w_i = softmax(scores_i, axis=-1)
    o_i = v_i @ w_i.T ;  cat = concat_i o_i     (T, 1024)
    out[b,h] = cat @ Wf.T + bf                  (T, 1024)

Because attention is over the feature dim, everything collapses through a
65x65 Gram matrix G = xa.T @ xa (xa = [xs, 1]):
    P         = G @ W~k_all                       (65, 1024)
    scT chunk = P_chunk.T @ W~q chunk  -> diagonal 64x64 blocks are
                scores_i^T (softmax axis lands on the psum partition dim)
    M~_i      = exp(scT_i).T @ [Wv_aug_i | bv | 1] (last col = denominator)
    M_i       = M~_i * (1/denom) per row
    N         = M.T @ Wf.T + u64 x bf             (65, 1024)
    out[b,h]  = xa @ N
This cuts FLOPs ~10x vs the naive dataflow. |scores| < ~50 so exp needs no
max-subtraction (f32 psum, bf16 storage).

Sharding: 32 independent (b, h) slices; 8 cores x 4 slices. Core c takes
b = c//4 and heads 4*(c%4)..4*(c%4)+3 so its x columns are contiguous.
Weights replicated, no collectives. Emission is software-pipelined: the
small-matmul stages (G/P/scores/M) of the next slice are striped between
the big N/out matmuls of the current slice so the tensor engine keeps a
high duty cycle (the PE queue is in-order, so striped work must only
depend on resources the running tail does not hold).

Key empirical costs (measured): a matmul whose stationary operand changed
pays ~110ns of serialized LDWEIGHTS on top of N/2.4GHz streaming, so loops
are ordered to reuse each stationary operand twice (both 512-halves).
Output is stored fp16, partition-major [j, p, c, :] for 4KB-contiguous
DMA descriptor runs; the host transposes back and upcasts.
"""

import numpy as np
import ml_dtypes

B, T, E, H = 2, 1024, 1024, 16
D = E // H
SCALE = float(np.sqrt(D))
NCORES = 8

_CACHE = {}


def _build_nc():
    from contextlib import ExitStack

    import concourse.bass as bass
    import concourse.mybir as mybir
    import concourse.tile as tile
    from concourse import bacc

    dt = mybir.dt
    AF = mybir.ActivationFunctionType

    nc = bacc.Bacc(None)
    xh_d = nc.declare_dram_parameter("xh", [128, 8, 4, 65], dt.float16, False)
    xt_d = nc.declare_dram_parameter("xt", [4, 65, 1024], dt.float16, False)
    wqt_d = nc.declare_dram_parameter("wqt", [65, 1024], dt.float16, False)
    wkt_d = nc.declare_dram_parameter("wkt", [65, 1024], dt.float16, False)
    wva_d = nc.declare_dram_parameter("wva", [128, 16, 66], dt.bfloat16, False)
    wft_d = nc.declare_dram_parameter("wft", [128, 8, 1024], dt.float16, False)
    bfh_d = nc.declare_dram_parameter("bfh", [1, 1024], dt.float16, False)
    ub_d = nc.declare_dram_parameter("ub", [1, 65], dt.float16, False)
    # out stored partition-major: out_d[j, p, c, :] = out row c*128+p. Gives
    # 4KB-contiguous per-partition DMA runs; host transposes back.
    out_d = nc.declare_dram_parameter("out", [4, 128, 8, 1024], dt.float16, True)

    with ExitStack() as ctx:
        tc = ctx.enter_context(tile.TileContext(nc))
        consts = ctx.enter_context(tc.tile_pool(name="consts", bufs=1))
        sbp = ctx.enter_context(tc.tile_pool(name="sbp", bufs=2))
        outp = ctx.enter_context(tc.tile_pool(name="outp", bufs=8))
        # PSUM (8 banks): ps_o 2x(128,1024)f32 = 4, ps_w 2 {warm, gps, pps,
        # nsp}, ps_s 2 {scp, mps} (v2-proven split: the M-stage ring and the
        # G/P/N ring stay independent so striped work rarely blocks the
        # in-order PE queue).
        ps_o = ctx.enter_context(tc.tile_pool(name="ps_o", bufs=2, space="PSUM"))
        ps_w = ctx.enter_context(tc.tile_pool(name="ps_w", bufs=2, space="PSUM"))
        ps_s = ctx.enter_context(tc.tile_pool(name="ps_s", bufs=2, space="PSUM"))

        # const DMAs: few big blobs (each dma_start costs ~750ns of issue on
        # its HWDGE queue), split across the sync and scalar queues so issue
        # overlaps. wft in 2 row-halves so the N-stage chases the stream.
        xh = consts.tile([128, 8, 4, 65], dt.float16, name="xh")
        nc.sync.dma_start(out=xh[:, 0:4], in_=xh_d[:, 0:4, :, :])
        nc.sync.dma_start(out=xh[:, 4:8], in_=xh_d[:, 4:8, :, :])
        wkt = consts.tile([65, 1024], dt.float16, name="wkt")
        nc.sync.dma_start(out=wkt[:], in_=wkt_d[:, :])
        wqt = consts.tile([65, 1024], dt.float16, name="wqt")
        nc.sync.dma_start(out=wqt[:], in_=wqt_d[:, :])
        wva = consts.tile([128, 16, 66], dt.bfloat16, name="wva")
        nc.sync.dma_start(out=wva[:], in_=wva_d[:, :, :])
        bfh = consts.tile([1, 1024], dt.float16, name="bfh")
        nc.sync.dma_start(out=bfh[:], in_=bfh_d[:, :])
        ub = consts.tile([1, 65], dt.float16, name="ub")
        nc.sync.dma_start(out=ub[:], in_=ub_d[:, :])
        xt = consts.tile([65, 4, 1024], dt.float16, name="xt")
        nc.sync.dma_start(out=xt[:, 0, :], in_=xt_d[0, :, :])
        wft = consts.tile([128, 8, 1024], dt.float16, name="wft")
        for k in range(4):
            nc.sync.dma_start(
                out=wft[:, 2 * k : 2 * k + 2], in_=wft_d[:, 2 * k : 2 * k + 2, :]
            )
        for j in range(1, 4):
            nc.sync.dma_start(out=xt[:, j, :], in_=xt_d[j, :, :])

        # PE warmup: dummy matmuls keep the HAM activity window busy while
        # the first input DMAs land.
        warm = consts.tile([128, 512], dt.float16, name="warm")
        nc.vector.memset(warm[:], 0.0)
        wps = ps_w.tile([128, 512], dt.float32, name="warmps", tag="psw")
        for _ in range(10):
            nc.tensor.matmul(wps[:], warm[:, 0:128], warm[:], start=True, stop=True)

        gsb = {}
        psb = {}
        expC = {}
        rec = {}
        msb = {}
        nsb = {}

        def emit_head(*js):
            """G, P, scoresT+exp, M stages for the given slices.
            Yields between work items (PE-instruction groups)."""
            for j in js:
                gps = ps_w.tile([65, 65], dt.float32, name=f"gps_{j}", tag="psw")
                for c in range(8):
                    nc.tensor.matmul(
                        gps[:], xh[:, c, j, :], xh[:, c, j, :],
                        start=(c == 0), stop=(c == 7),
                    )
                    if c == 3:
                        yield
                gsb[j] = sbp.tile([65, 65], dt.float16, name=f"gsb_{j}", tag="gsb")
                nc.vector.tensor_copy(out=gsb[j][:], in_=gps[:])
                yield
            for j in js:
                psb[j] = sbp.tile([65, 1024], dt.float16, name=f"psb_{j}", tag="psb")
                pps0 = ps_w.tile([65, 512], dt.float32, name=f"pps_{j}_0", tag="psw")
                pps1 = ps_w.tile([65, 512], dt.float32, name=f"pps_{j}_1", tag="psw")
                nc.tensor.matmul(pps0[:], gsb[j][:], wkt[:, 0:512], start=True, stop=True)
                nc.tensor.matmul(pps1[:], gsb[j][:], wkt[:, 512:1024], start=True, stop=True)
                yield
                nc.vector.tensor_copy(out=psb[j][:, 0:512], in_=pps0[:])
                nc.scalar.copy(out=psb[j][:, 512:1024], in_=pps1[:])
                yield
            for j in js:
                # scT chunks: diag 64x64 blocks of P_chunk.T @ W~q_chunk
                expC[j] = sbp.tile([128, 8, 128], dt.bfloat16, name=f"expC_{j}", tag="expC")
                for t in range(2):
                    scp = ps_s.tile([128, 512], dt.float32, name=f"scp_{j}_{t}", tag="pss")
                    for u in range(4):
                        c = 4 * t + u
                        nc.tensor.matmul(
                            scp[:, u * 128 : (u + 1) * 128],
                            psb[j][:, c * 128 : (c + 1) * 128],
                            wqt[:, c * 128 : (c + 1) * 128],
                            start=True, stop=True,
                        )
                        if u == 1:
                            yield
                    nc.scalar.activation(
                        out=expC[j][:, 4 * t : 4 * t + 4, :], in_=scp[:], func=AF.Exp
                    )
                    yield
            for j in js:
                rec[j] = sbp.tile([128, 8], dt.float32, name=f"rec_{j}", tag="rec")
                msb[j] = sbp.tile([128, 8, 65], dt.float16, name=f"msb_{j}", tag="msb")
                for c in range(8):
                    mps = ps_s.tile([128, 66], dt.float32, name=f"mps_{j}_{c}", tag="pss")
                    nc.tensor.matmul(
                        mps[0:64, :], expC[j][0:64, c, 0:64], wva[0:64, 2 * c, :],
                        start=True, stop=True,
                    )
                    nc.tensor.matmul(
                        mps[64:128, :], expC[j][64:128, c, 64:128], wva[64:128, 2 * c + 1, :],
                        start=True, stop=True,
                    )
                    nc.vector.reciprocal(out=rec[j][:, c : c + 1], in_=mps[:, 65:66])
                    nc.vector.tensor_scalar_mul(
                        out=msb[j][:, c, :], in0=mps[:, 0:65], scalar1=rec[j][:, c : c + 1]
                    )
                    yield

        def emit_tail(*js):
            """N and out stages for the given slices."""
            for j in js:
                nsb[j] = sbp.tile([65, 1024], dt.float16, name=f"nsb_{j}", tag="nsb")
                for nh in range(2):
                    nsp = ps_w.tile([65, 512], dt.float32, name=f"nsp_{j}_{nh}", tag="psw")
                    for c in range(8):
                        nc.tensor.matmul(
                            nsp[:], msb[j][:, c, :], wft[:, c, nh * 512 : (nh + 1) * 512],
                            start=(c == 0), stop=False,
                        )
                        if c % 2 == 1:
                            yield
                    nc.tensor.matmul(
                        nsp[:], ub[:], bfh[:, nh * 512 : (nh + 1) * 512],
                        start=False, stop=True,
                    )
                    if nh == 0:
                        nc.vector.tensor_copy(out=nsb[j][:, 0:512], in_=nsp[:])
                    else:
                        nc.scalar.copy(out=nsb[j][:, 512:1024], in_=nsp[:])
                    yield
            for j in js:
                for c in range(8):
                    if c % 2 == 0:
                        osb = outp.tile([128, 2, 1024], dt.float16, name=f"osb_{j}_{c}", tag="osb")
                    ops = ps_o.tile([128, 1024], dt.float32, name=f"ops_{j}_{c}", tag="pso")
                    for nh in range(2):
                        nc.tensor.matmul(
                            ops[:, nh * 512 : (nh + 1) * 512],
                            xt[:, j, c * 128 : (c + 1) * 128],
                            nsb[j][:, nh * 512 : (nh + 1) * 512],
                            start=True, stop=True,
                        )
                    yield
                    if c % 2 == 0:
                        nc.vector.tensor_copy(out=osb[:, 0, :], in_=ops[:])
                    else:
                        nc.scalar.copy(out=osb[:, 1, :], in_=ops[:])
                    yield
                    if c % 2 == 1:
                        nc.sync.dma_start(
                            out=out_d[j, :, c - 1 : c + 1, :], in_=osb[:]
                        )

        def drain(gen):
            for _ in gen:
                pass

        def stripe(a, b):
            a_live, b_live = True, True
            while a_live or b_live:
                if a_live:
                    a_live = next(a, _SENT) is not _SENT
                if b_live:
                    b_live = next(b, _SENT) is not _SENT

        # software pipeline at slice granularity: every tail (big, copy-heavy)
        # is striped with the next slice's head (small matmuls) so the PE
        # always has dense work and the copy engines drain in parallel.
        drain(emit_head(0))
        for s in range(4):
            if s < 3:
                stripe(emit_tail(s), emit_head(s + 1))
            else:
                drain(emit_tail(s))

    nc.finalize()
    return nc


_SENT = object()


def _prep_weights(Wq, bq, Wk, bk, Wv, bv, Wf, bf):
    wqt = np.zeros((65, 1024), np.float16)
    wqt[:64] = (np.transpose(Wq, (2, 0, 1)).reshape(64, H * D) / SCALE).astype(np.float16)
    wqt[64] = (bq.reshape(H * D) / SCALE).astype(np.float16)
    wkt = np.zeros((65, 1024), np.float16)
    wkt[:64] = np.transpose(Wk, (2, 0, 1)).reshape(64, H * D).astype(np.float16)
    wkt[64] = bk.reshape(H * D).astype(np.float16)
    wva_h = np.zeros((64, 16, 66), ml_dtypes.bfloat16)
    wva_h[:, :, :64] = np.transpose(Wv, (1, 0, 2)).astype(ml_dtypes.bfloat16)
    wva_h[:, :, 64] = bv.T.astype(ml_dtypes.bfloat16)
    wva_h[:, :, 65] = 1.0
    wva = np.concatenate([wva_h, wva_h], axis=0)  # duplicated for row-base-64 matmuls
    wft = np.ascontiguousarray(
        Wf.T.reshape(8, 128, 1024).transpose(1, 0, 2)
    ).astype(np.float16)
    bfh = bf.reshape(1, 1024).astype(np.float16)
    ub = np.zeros((1, 65), np.float16)
    ub[0, 64] = 1.0
    return wqt, wkt, wva, wft, bfh, ub


def _prep_x(xs):
    """xs (1024, 256) f32 -> xh (128, 8, 4, 65) fp16 with ones col,
    xt (4, 65, 1024) fp16 with ones row."""
    x16 = xs.astype(np.float16)
    xh = np.ones((128, 8, 4, 65), np.float16)
    xh[:, :, :, :64] = x16.reshape(8, 128, 4, 64).transpose(1, 0, 2, 3)
    xt = np.ones((4, 65, 1024), np.float16)
    xt[:, :64] = x16.reshape(1024, 4, 64).transpose(1, 2, 0)
    return xh, xt


def _run(inputs, trace=False, tmpdir=None):
    from concourse.bass_utils import run_bass_kernel_spmd

    if "nc" not in _CACHE:
        _CACHE["nc"] = _build_nc()
    nc = _CACHE["nc"]

    x = np.ascontiguousarray(np.asarray(inputs["x"]), dtype=np.float32)
    wqt, wkt, wva, wft, bfh, ub = _prep_weights(
        *(np.asarray(inputs[k], dtype=np.float32) for k in
          ("Wq", "bq", "Wk", "bk", "Wv", "bv", "Wf", "bf"))
    )
    common = dict(wqt=wqt, wkt=wkt, wva=wva, wft=wft, bfh=bfh, ub=ub)
    in_maps = []
    for c in range(NCORES):
        xs = np.ascontiguousarray(x[c // 4][:, (c % 4) * 256 : (c % 4 + 1) * 256])
        xhc, xtc = _prep_x(xs)
        in_maps.append(dict(xh=xhc, xt=xtc, **common))

    res = run_bass_kernel_spmd(
        nc, in_maps, list(range(NCORES)), trace=trace, tmpdir=tmpdir
    )
    out = np.empty((B, H, T, E), np.float32)
    for c in range(NCORES):
        oc = res.results[c]["out"]  # (4, 128, 8, 1024): [j, p, cblk, :]
        oc = np.transpose(oc, (0, 2, 1, 3)).reshape(4, T, E)
        out[c // 4, 4 * (c % 4) : 4 * (c % 4) + 4] = oc.astype(np.float32)
    return out, res.exec_time_ns


def kernel(**inputs) -> np.ndarray:
    out, _ = _run(inputs, trace=False)
    return out



# revision 6
# speedup vs baseline: 1.1354x; 1.1354x over previous
"""Trainium2 Bass kernel for nn_MultiHeadSelfAttention_55654186222044.

Reference math (per batch b, per "slice" h of the reshaped activations):
    xs  = x[b,:,h*64:(h+1)*64]                  (T=1024, D=64)
    q_i = xs @ Wq[i].T + bq[i]   (per param set i=0..15), same k_i, v_i
    scores_i = q_i.T @ k_i / 8   (64x64, contraction over T!)
    w_i = softmax(scores_i, axis=-1)
    o_i = v_i @ w_i.T ;  cat = concat_i o_i     (T, 1024)
    out[b,h] = cat @ Wf.T + bf                  (T, 1024)

Because attention is over the feature dim, everything collapses through a
65x65 Gram matrix G = xa.T @ xa (xa = [xs, 1]):
    P      = G @ W~k_all                         (65, 1024)
    scT_c  = P_c.T @ W~q_c   (128-col chunks)    diag 64x64 blocks = scores_i^T
             (softmax axis lands on the psum partition dim)
    exp    -> expC, stored into a PRE-ZEROED (128,8,128) tile so that each
             expC[:,c,:] is the block-diagonal [exp_2c, exp_2c+1]
    M~_c   = expC[:,c,:].T @ wva2[:,c,:]  (one matmul per chunk; col 65 of
             wva2 is ones so col 65 of M~ is the softmax denominator)
    M      = M~ * (1/denom) per row; N = M.T @ Wf.T + u x bf   (65, 1024)
    out[b,h] = xa @ N
This cuts FLOPs ~10x vs the naive dataflow. |scores| < ~50 so exp needs no
max-subtraction (f32 psum, bf16 expC storage - e^50 overflows fp16).

Schedule (v2): warmup matmuls ramp the PE clock while input DMAs stream on
the sync queue in priority order. All four slices' small stages ("heads")
run first, overlapping the input DMA; then eight "half-tails" (N-half then
out-half per slice, 512 output columns at a time) keep the tensor engine
gapless at full clock. Evictions (psum->sbuf fp16 casts) round-robin over
vector/gpsimd/scalar; output DMAs are issued from the vector/scalar queues
in 0.5MB chunks so the 8.4MB output streams out concurrently with compute.

Sharding: 32 independent (b, h) slices; 8 cores x 4 slices. Core c takes
b = c//4 and heads 4*(c%4)..4*(c%4)+3 so its x columns are contiguous.
Weights replicated, no collectives. Output is stored fp16, partition-major
[j, p, c, :]; the host transposes back and upcasts.
"""

import numpy as np
import ml_dtypes

B, T, E, H = 2, 1024, 1024, 16
D = E // H
SCALE = float(np.sqrt(D))
NCORES = 8

_CACHE = {}


def _build_nc():
    from contextlib import ExitStack

    import concourse.bass as bass
    import concourse.mybir as mybir
    import concourse.tile as tile
    from concourse import bacc

    dt = mybir.dt
    AF = mybir.ActivationFunctionType

    nc = bacc.Bacc(None)
    xh_d = nc.declare_dram_parameter("xh", [128, 4, 8, 65], dt.float16, False)
    xt_d = nc.declare_dram_parameter("xt", [65, 4, 1024], dt.float16, False)
    wqk_d = nc.declare_dram_parameter("wqk", [65, 2048], dt.float16, False)
    ubf_d = nc.declare_dram_parameter("ubf", [1, 1089], dt.float16, False)
    wva2_d = nc.declare_dram_parameter("wva2", [128, 8, 66], dt.bfloat16, False)
    wft_d = nc.declare_dram_parameter("wft", [128, 8, 1024], dt.float16, False)
    # out stored partition-major: out_d[j, p, c, :] = out row c*128+p of slice
    # j. 1KB-contiguous per-partition DMA descriptor runs; host transposes.
    out_d = nc.declare_dram_parameter("out", [4, 128, 8, 1024], dt.float16, True)

    with ExitStack() as ctx:
        tc = ctx.enter_context(tile.TileContext(nc))
        consts = ctx.enter_context(tc.tile_pool(name="consts", bufs=1))
        sbp = ctx.enter_context(tc.tile_pool(name="sbp", bufs=2))
        msp = ctx.enter_context(tc.tile_pool(name="msp", bufs=4))
        outp = ctx.enter_context(tc.tile_pool(name="outp", bufs=4))
        # PSUM (8 banks): ph 3 (head ring: gps/pps/scp/mall), pn 2 (N halves),
        # po 3 (out-stage ring + warmup/bridge dummies).
        ph = ctx.enter_context(tc.tile_pool(name="ph", bufs=3, space="PSUM"))
        pn = ctx.enter_context(tc.tile_pool(name="pn", bufs=2, space="PSUM"))
        po = ctx.enter_context(tc.tile_pool(name="po", bufs=3, space="PSUM"))

        # ---- input DMAs, priority order, all on the sync HWDGE queue ----
        xh = consts.tile([128, 4, 8, 65], dt.float16, name="xh")
        nc.sync.dma_start(out=xh[:, 0], in_=xh_d[:, 0])
        wqk = consts.tile([65, 2048], dt.float16, name="wqk")
        nc.sync.dma_start(out=wqk[:], in_=wqk_d[:, :])
        ubf = consts.tile([1, 1089], dt.float16, name="ubf")
        nc.sync.dma_start(out=ubf[:], in_=ubf_d[:, :])
        nc.sync.dma_start(out=xh[:, 1:4], in_=xh_d[:, 1:4])
        wva2 = consts.tile([128, 8, 66], dt.bfloat16, name="wva2")
        nc.sync.dma_start(out=wva2[:], in_=wva2_d[:, :, :])
        wft = consts.tile([128, 8, 1024], dt.float16, name="wft")
        nc.sync.dma_start(out=wft[:, :, 0:512], in_=wft_d[:, :, 0:512])
        nc.sync.dma_start(out=wft[:, :, 512:1024], in_=wft_d[:, :, 512:1024])
        xt = consts.tile([65, 4, 1024], dt.float16, name="xt")
        nc.sync.dma_start(out=xt[:], in_=xt_d[:, :, :])

        # ---- PE warmup: ramp the clock while xh[0]/wqk land ----
        warm = consts.tile([128, 512], dt.float16, name="warm")
        nc.vector.memset(warm[:], 0.0)
        wps = po.tile([128, 512], dt.float32, name="warm_ps", tag="po")
        for _ in range(12):
            nc.tensor.matmul(wps[:, 0:128], warm[:, 0:128], warm[:, 0:128],
                             start=True, stop=True)

        def bridge(n, nm):
            bps = po.tile([128, 512], dt.float32, name=f"br_{nm}", tag="po")
            for k in range(n):
                nc.tensor.matmul(bps[:, 0:128], warm[:, 0:128], warm[:, 0:128],
                                 start=True, stop=True)

        # expC ring (2 buffers): pre-zero both; only diagonal 64-blocks are
        # ever rewritten, so off-diagonal zeros persist across slices.
        expC = {}
        for j in range(4):
            expC[j] = sbp.tile([128, 8, 128], dt.bfloat16, name=f"expC_{j}",
                               tag="expC")
        nc.vector.memset(expC[0][:], 0.0)
        nc.gpsimd.memset(expC[1][:], 0.0)

        wqt = wqk[0:65, 0:1024]
        wkt = wqk[0:65, 1024:2048]
        bfh = ubf[0:1, 0:1024]
        ub = ubf[0:1, 1024:1089]

        msb = {}
        nsb = {}

        def head(j):
            # G = xa.T @ xa  (65, 65)
            gps = ph.tile([65, 65], dt.float32, name=f"gps_{j}", tag="ph")
            for c in range(8):
                nc.tensor.matmul(gps[:], xh[:, j, c, :], xh[:, j, c, :],
                                 start=(c == 0), stop=(c == 7))
            if j == 0:
                bridge(3, "p0")  # wqk may land just after G0
            gsb = sbp.tile([65, 65], dt.float16, name=f"gsb_{j}", tag="gsb")
            nc.vector.tensor_copy(out=gsb[:], in_=gps[:])
            # P = G @ W~k  (65, 1024)
            psb = sbp.tile([65, 1024], dt.float16, name=f"psb_{j}", tag="psb")
            pps0 = ph.tile([65, 512], dt.float32, name=f"pps_{j}_0", tag="ph")
            pps1 = ph.tile([65, 512], dt.float32, name=f"pps_{j}_1", tag="ph")
            nc.tensor.matmul(pps0[:], gsb[:], wkt[:, 0:512], start=True, stop=True)
            nc.tensor.matmul(pps1[:], gsb[:], wkt[:, 512:1024], start=True, stop=True)
            nc.scalar.copy(out=psb[:, 0:512], in_=pps0[:])
            nc.vector.tensor_copy(out=psb[:, 512:1024], in_=pps1[:])
            # scores^T chunks + exp into block-diag layout
            for t in range(2):
                scp = ph.tile([128, 4, 128], dt.float32, name=f"scp_{j}_{t}",
                              tag="ph")
                for u in range(4):
                    c = 4 * t + u
                    nc.tensor.matmul(
                        scp[:, u, :],
                        psb[:, c * 128:(c + 1) * 128],
                        wqt[:, c * 128:(c + 1) * 128],
                        start=True, stop=True,
                    )
                nc.scalar.activation(
                    out=expC[j][0:64, 4 * t:4 * t + 4, 0:64],
                    in_=scp[0:64, :, 0:64], func=AF.Exp)
                nc.scalar.activation(
                    out=expC[j][64:128, 4 * t:4 * t + 4, 64:128],
                    in_=scp[64:128, :, 64:128], func=AF.Exp)
            # M~ = expC.T @ wva2 per chunk (block-diag, one matmul each);
            # col 65 is the softmax denominator.
            rec = msp.tile([128, 8], dt.float32, name=f"rec_{j}", tag="rec")
            msb[j] = msp.tile([128, 8, 65], dt.float16, name=f"msb_{j}", tag="msb")
            mall = []
            for half in range(2):
                mps = ph.tile([128, 4, 66], dt.float32, name=f"mps_{j}_{half}",
                              tag="ph")
                mall.append(mps)
                for u in range(4):
                    c = 4 * half + u
                    nc.tensor.matmul(mps[:, u, :], expC[j][:, c, :],
                                     wva2[:, c, :], start=True, stop=True)
                nc.vector.reciprocal(out=rec[:, 4 * half:4 * half + 4],
                                     in_=mps[:, :, 65])
            for c in range(8):
                if c % 2 == 0:
                    nc.vector.tensor_scalar_mul(
                        out=msb[j][:, c, :], in0=mall[c // 4][:, c % 4, 0:65],
                        scalar1=rec[:, c:c + 1])
                else:
                    nc.scalar.activation(
                        out=msb[j][:, c, :], in_=mall[c // 4][:, c % 4, 0:65],
                        func=AF.Copy, scale=rec[:, c:c + 1])

        def tail_N(j, nh):
            # N half: (65, 512) = sum_c msb_c.T @ wft_c + u x bf
            nsp = pn.tile([65, 512], dt.float32, name=f"nsp_{j}_{nh}", tag="pn")
            lo = nh * 512
            for c in range(8):
                nc.tensor.matmul(nsp[:], msb[j][:, c, :], wft[:, c, lo:lo + 512],
                                 start=(c == 0), stop=False)
            nc.tensor.matmul(nsp[:], ub, bfh[:, lo:lo + 512],
                             start=False, stop=True)
            if nh == 0:
                nsb[j] = sbp.tile([65, 1024], dt.float16, name=f"nsb_{j}",
                                  tag="nsb")
                nc.vector.tensor_copy(out=nsb[j][:, 0:512], in_=nsp[:])
            else:
                nc.scalar.copy(out=nsb[j][:, 512:1024], in_=nsp[:])

        EV = None

        def tail_out(j, nh):
            # out half: (1024, 512) = xa @ N[:, half], 8 row chunks
            lo = nh * 512
            osb = None
            for c in range(8):
                if c % 4 == 0:
                    osb = outp.tile([128, 4, 512], dt.float16,
                                    name=f"osb_{j}_{nh}_{c // 4}", tag="osb")
                ops = po.tile([128, 512], dt.float32, name=f"ops_{j}_{nh}_{c}",
                              tag="po")
                nc.tensor.matmul(ops[:], xt[:, j, c * 128:(c + 1) * 128],
                                 nsb[j][:, lo:lo + 512], start=True, stop=True)
                if (8 * nh + c) % 2 == 0:
                    nc.vector.tensor_copy(out=osb[:, c % 4, :], in_=ops[:])
                else:
                    nc.scalar.copy(out=osb[:, c % 4, :], in_=ops[:])
                if c % 4 == 3:
                    nc.sync.dma_start(
                        out=out_d[j, :, c - 3:c + 1, lo:lo + 512], in_=osb[:])

        for j in range(4):
            head(j)
        for j in range(4):
            tail_N(j, 0)
            if j == 0:
                bridge(8, "w1")  # wft second half may land just after N0-h0
            tail_N(j, 1)
            tail_out(j, 0)
            tail_out(j, 1)

    nc.finalize()
    return nc


def _prep_weights(Wq, bq, Wk, bk, Wv, bv, Wf, bf):
    wqk = np.zeros((65, 2048), np.float16)
    wqk[:64, 0:1024] = (np.transpose(Wq, (2, 0, 1)).reshape(64, H * D) / SCALE
                        ).astype(np.float16)
    wqk[64, 0:1024] = (bq.reshape(H * D) / SCALE).astype(np.float16)
    wqk[:64, 1024:2048] = np.transpose(Wk, (2, 0, 1)).reshape(64, H * D
                                                              ).astype(np.float16)
    wqk[64, 1024:2048] = bk.reshape(H * D).astype(np.float16)
    ubf = np.zeros((1, 1089), np.float16)
    ubf[0, 0:1024] = bf.astype(np.float16)
    ubf[0, 1024 + 64] = 1.0
    wva_h = np.zeros((64, 16, 66), np.float32)
    wva_h[:, :, :64] = np.transpose(Wv, (1, 0, 2))
    wva_h[:, :, 64] = bv.T
    wva_h[:, :, 65] = 1.0
    wva2 = np.zeros((128, 8, 66), np.float32)
    wva2[0:64] = wva_h[:, 0::2, :]
    wva2[64:128] = wva_h[:, 1::2, :]
    wva2 = wva2.astype(ml_dtypes.bfloat16)
    wft = np.ascontiguousarray(
        Wf.T.reshape(8, 128, 1024).transpose(1, 0, 2)
    ).astype(np.float16)
    return wqk, ubf, wva2, wft


def _prep_x(xs):
    """xs (1024, 256) f32 -> xh (128, 4, 8, 65) fp16 with ones col,
    xt (65, 4, 1024) fp16 with ones row."""
    x16 = xs.astype(np.float16)
    xh = np.ones((128, 4, 8, 65), np.float16)
    xh[:, :, :, :64] = x16.reshape(8, 128, 4, 64).transpose(1, 2, 0, 3)
    xt = np.ones((65, 4, 1024), np.float16)
    xt[:64] = x16.reshape(1024, 4, 64).transpose(2, 1, 0)
    return xh, xt


def _run(inputs, trace=False, tmpdir=None):
    from concourse.bass_utils import run_bass_kernel_spmd

    if "nc" not in _CACHE:
        _CACHE["nc"] = _build_nc()
    nc = _CACHE["nc"]

    x = np.ascontiguousarray(np.asarray(inputs["x"]), dtype=np.float32)
    wqk, ubf, wva2, wft = _prep_weights(
        *(np.asarray(inputs[k], dtype=np.float32) for k in
          ("Wq", "bq", "Wk", "bk", "Wv", "bv", "Wf", "bf"))
    )
    common = dict(wqk=wqk, ubf=ubf, wva2=wva2, wft=wft)
    in_maps = []
    for c in range(NCORES):
        xs = np.ascontiguousarray(x[c // 4][:, (c % 4) * 256: (c % 4 + 1) * 256])
        xhc, xtc = _prep_x(xs)
        in_maps.append(dict(xh=xhc, xt=xtc, **common))

    res = run_bass_kernel_spmd(
        nc, in_maps, list(range(NCORES)), trace=trace, tmpdir=tmpdir
    )
    out = np.empty((B, H, T, E), np.float32)
    for c in range(NCORES):
        oc = res.results[c]["out"]  # (4, 128, 8, 1024): [j, p, cblk, :]
        oc = np.transpose(oc, (0, 2, 1, 3)).reshape(4, T, E)
        out[c // 4, 4 * (c % 4): 4 * (c % 4) + 4] = oc.astype(np.float32)
    return out, res.exec_time_ns


def kernel(**inputs) -> np.ndarray:
    out, _ = _run(inputs, trace=False)
    return out
